# revision 2
# baseline (speedup 1.0000x reference)
"""Trainium2 Bass kernel for DifferentiableSparseHypergraph (topk_masking).

Full computation per batch n:
  x_mean = x[n].mean(T)                      (C, V)
  q = Wq @ x_mean + bq                       (O=32, V)   [1x1 conv == matmul]
  q = q / max(||q||_2 over O, eps)
  H_raw = (q^T @ key_prototypes) / sqrt(O)   (V, M=128)
  topk10 -> softmax over the 10 vals -> scatter back; zeros elsewhere.

Kernel strategy (pure data-parallel over batch, 8 cores x 8 batches):
  * t-mean: ONE DVE add level pairs t's (64 -> 32), then the remaining
    32-way t-sum rides the PE: 8 fp32 matmuls (4 free-chunks x 2 c-halves)
    accumulate into a single [32, 512] PSUM tile; slot (tl, v) collects
    t-pairs {tl, 8+tl, 16+tl, 24+tl}. A strided DVE reduce (8 -> 1) and an
    ACT bias/scale step produce q. This moves ~45 us of fp32 tensor_tensor
    work (1x-rate on DVE) onto the 70%-idle PE at 4 cyc/row.
  * top-10 runs scale-invariantly on the RAW score matmul pb (still in
    PSUM): per-row ordering of H = pb * rn (rn > 0) equals ordering of pb,
    so max/match_replace/max finds the 10th-largest threshold t_k without
    waiting for the norm. exp(H) fuses the rn scale into ACT's Exp; the
    output is exp * (pb >= t_k) / sum -- identical to softmax-over-topk
    scattered back.
  * batches 0 and 7 stream in t-range chunks so compute starts right as
    the first MiB lands and the tail after the last byte stays ~6 us.
"""

import numpy as np

import concourse.bacc as bacc
import concourse.bass as bass
import concourse.mybir as mybir
import concourse.tile as tile

N, C, T, V = 64, 256, 64, 64
INTER = 32          # conv out channels
M = 128             # num hyperedges
TOPK = 10
NCORES = 8
FP = mybir.dt.float32
NEG_BIG = -1.0e30


def build_nc(nloc: int) -> bass.Bass:
    """Build the per-core Bass program processing `nloc` batches."""
    assert nloc % 2 == 0
    # Bacc (not bare Bass): its compile()/finalize() pipeline splits
    # multi-semaphore waits into InstEventSemaphore pairs — walrus allows
    # at most one sync wait per regular instruction.
    nc = bacc.Bacc(target_bir_lowering=False, debug=False)

    x = nc.dram_tensor("x", (nloc, C, T, V), FP, kind="ExternalInput")
    wqt = nc.dram_tensor("wqt", (C, INTER), FP, kind="ExternalInput")
    kp = nc.dram_tensor("kp", (INTER, M), FP, kind="ExternalInput")
    bq = nc.dram_tensor("bq", (INTER, 1), FP, kind="ExternalInput")
    out = nc.dram_tensor("out", (nloc, V, M), FP, kind="ExternalOutput")

    A = mybir.AluOpType
    AF = mybir.ActivationFunctionType
    from concourse.tile import add_dep_helper

    last = nloc - 1

    with tile.TileContext(nc) as tc:
        with (
            tc.tile_pool(name="consts", bufs=1) as consts,
            tc.tile_pool(name="xph", bufs=3) as xph,
            tc.tile_pool(name="xp", bufs=2) as xp,
            tc.tile_pool(name="small", bufs=2) as small,
            tc.tile_pool(name="psA", bufs=2, space="PSUM") as psA,
            tc.tile_pool(name="psB", bufs=2, space="PSUM") as psB,
            tc.tile_pool(name="psS", bufs=1, space="PSUM") as psS,
        ):
            # --- batch 0's x loads go FIRST (before the const DMAs) and in
            # 1 MiB t-range chunks, so the DVE adds start as early as
            # possible.  Chunked batches pair t with t+1 (each chunk is
            # self-contained); whole-half batches pair t with t+32.
            xh0 = [
                xph.tile([128, T * V], FP, tag=f"xh{h}", name=f"xh0_{h}")
                for h in range(2)
            ]
            for h in range(2):
                for c in range(2):
                    nc.sync.dma_start(
                        out=xh0[h][:, c * 2048 : (c + 1) * 2048],
                        in_=x[0, h * 128 : (h + 1) * 128,
                              c * (T // 2) : (c + 1) * (T // 2)],
                    )

            # --- replicated constants ---
            wq_sb = consts.tile([128, 2, INTER], FP)    # [c, c_half, o]
            nc.sync.dma_start(
                out=wq_sb[:], in_=wqt.rearrange("(h c) o -> c h o", h=2)
            )
            kp_sb = consts.tile([INTER, M], FP)
            nc.sync.dma_start(out=kp_sb[:], in_=kp[:])
            bq_sb = consts.tile([INTER, 1], FP)
            nc.sync.dma_start(out=bq_sb[:], in_=bq[:])
            ones_sb = consts.tile([INTER, 1], FP)
            nc.vector.memset(ones_sb[:], 1.0)

            # The fp32 self-loading matmul can carry at most ONE semaphore
            # wait (walrus S3_LW_STRUCT limit). Absorb the wq/kp DMA waits
            # with dummy 1x1 matmuls so the first real matmuls only wait on
            # their a1-tile DVE sem.
            scr = psS.tile([1, 1], FP)
            d1 = nc.tensor.matmul(
                scr[:], wq_sb[:, 0, 0:1], wq_sb[:, 0, 0:1], start=True, stop=True
            )
            d2 = nc.tensor.matmul(
                scr[:], kp_sb[:, 0:1], kp_sb[:, 0:1], start=True, stop=True
            )
            add_dep_helper(d2.ins, d1.ins, sync=False, reason="pe-wait-absorb order")

            q2 = None
            first_mm = None
            for n in range(nloc):
                chunked = n == 0 or n == last
                if n == 0:
                    xh = xh0
                else:
                    xh = []
                    for h in range(2):
                        t = xph.tile([128, T * V], FP, tag=f"xh{h}")
                        if chunked:
                            # last batch: 4 t-range chunks per half so the
                            # post-last-byte tail is one chunk, not a half
                            for c in range(4):
                                nc.sync.dma_start(
                                    out=t[:, c * 1024 : (c + 1) * 1024],
                                    in_=x[n, h * 128 : (h + 1) * 128,
                                          c * (T // 4) : (c + 1) * (T // 4)],
                                )
                        else:
                            nc.sync.dma_start(
                                out=t[:], in_=x[n, h * 128 : (h + 1) * 128]
                            )
                        xh.append(t)

                # one DVE add level: t 64 -> 32 (a1 free = 32 t-slots x V),
                # then 8 accumulating matmuls fold the 32 slots into 8 PSUM
                # slots (tl, v) while contracting c.
                l = n % 2
                if l == 0:
                    q2 = small.tile([INTER, 2 * V], FP, tag="q2")
                pa = psA.tile([INTER, 512], FP, tag="pa")
                mm_idx = 0
                for h in range(2):
                    a1 = xp.tile([128, T * V // 2], FP, tag=f"a1{h}")
                    if chunked:
                        nch = 2 if n == 0 else 4
                        csz = 4096 // nch
                        for c in range(nch):
                            src = xh[h][
                                :, c * csz : (c + 1) * csz
                            ].rearrange("p (t two v) -> p t two v", two=2, v=V)
                            dst = a1[
                                :, c * (csz // 2) : (c + 1) * (csz // 2)
                            ].rearrange("p (t v) -> p t v", v=V)
                            nc.vector.tensor_add(
                                dst, src[:, :, 0, :], src[:, :, 1, :]
                            )
                            # matmul each completed 512-span of a1 right away
                            done = (c + 1) * (csz // 2)
                            while mm_idx * 512 + 512 <= h * 2048 + done:
                                j = mm_idx - h * 4
                                mm = nc.tensor.matmul(
                                    pa[:],
                                    wq_sb[:, h, :],
                                    a1[:, j * 512 : (j + 1) * 512],
                                    start=(mm_idx == 0),
                                    stop=(mm_idx == 7),
                                )
                                if first_mm is None:
                                    first_mm = mm
                                    add_dep_helper(
                                        mm.ins, d2.ins, sync=False,
                                        reason="pe-wait-absorb order",
                                    )
                                mm_idx += 1
                    else:
                        nc.vector.tensor_add(
                            a1[:],
                            xh[h][:, : T * V // 2],
                            xh[h][:, T * V // 2 :],
                        )
                        for j in range(4):
                            nc.tensor.matmul(
                                pa[:],
                                wq_sb[:, h, :],
                                a1[:, j * 512 : (j + 1) * 512],
                                start=(mm_idx == 0),
                                stop=(mm_idx == 7),
                            )
                            mm_idx += 1

                # 8 psum slots -> q (strided DVE reduce), then +bq, /T on ACT
                qtmp = small.tile([INTER, V], FP, tag="qtmp")
                nc.vector.reduce_sum(
                    out=qtmp[:],
                    in_=pa[:].rearrange("o (t v) -> o v t", t=8),
                    axis=mybir.AxisListType.X,
                )
                nc.scalar.activation(
                    q2[:, l * V : (l + 1) * V],
                    qtmp[:],
                    AF.Identity,
                    bias=bq_sb[:],
                    scale=1.0 / T,
                )
                if l == 0:
                    continue
                p = n // 2

                # raw scores pb[vv, m] = q2^T . kp stay in PSUM; the top-10
                # threshold is found on pb directly (ordering-invariant to
                # the positive per-row rescale rn).
                qsq = small.tile([INTER, 2 * V], FP, tag="qsq")
                nc.scalar.activation(qsq[:], q2[:], AF.Square)
                pb = psB.tile([2 * V, M], FP, tag="pb")
                nc.tensor.matmul(pb[:], q2[:], kp_sb[:], start=True, stop=True)
                pc = psB.tile([2 * V, 1], FP, tag="pc")
                nc.tensor.matmul(pc[:], qsq[:], ones_sb[:], start=True, stop=True)
                # rn = 1/sqrt(INTER * nsq) = INTER^-0.5 / ||q||
                nrm = small.tile([2 * V, 1], FP, tag="nrm")
                nc.scalar.activation(nrm[:], pc[:], AF.Sqrt, scale=float(INTER))
                rn = small.tile([2 * V, 1], FP, tag="rn")
                nc.vector.reciprocal(rn[:], nrm[:])

                # t_k = 10th largest per row: top8, knock them out, top8 again
                top8a = small.tile([2 * V, 8], FP, tag="t8a")
                nc.vector.max(top8a[:], pb[:])
                work = small.tile([2 * V, M], FP, tag="work")
                nc.vector.match_replace(work[:], top8a[:], pb[:], NEG_BIG)
                top8b = small.tile([2 * V, 8], FP, tag="t8b")
                nc.vector.max(top8b[:], work[:])

                # e = exp(H) = exp(pb * rn)  (rn fused into ACT's scale);
                # masked softmax without scatter:
                # me = (pb >= t_k) * e; out = me / sum(me)
                e = small.tile([2 * V, M], FP, tag="e")
                nc.scalar.activation(e[:], pb[:], AF.Exp, scale=rn[:])
                me = small.tile([2 * V, M], FP, tag="me")
                s = small.tile([2 * V, 1], FP, tag="s")
                nc.vector.scalar_tensor_tensor(
                    out=me[:],
                    in0=pb[:],
                    scalar=top8b[:, 1:2],
                    in1=e[:],
                    op0=A.is_ge,
                    op1=A.mult,
                    accum_out=s[:],
                )
                r = small.tile([2 * V, 1], FP, tag="r")
                nc.vector.reciprocal(r[:], s[:])
                ot = small.tile([2 * V, M], FP, tag="ot")
                nc.scalar.activation(ot[:], me[:], AF.Copy, scale=r[:])

                nc.sync.dma_start(
                    out=out[2 * p : 2 * p + 2].rearrange("b v m -> (b v) m"),
                    in_=ot[:],
                )
    nc.finalize()
    return nc


_NC_CACHE: dict[int, bass.Bass] = {}


def _get_nc(nloc: int) -> bass.Bass:
    if nloc not in _NC_CACHE:
        _NC_CACHE[nloc] = build_nc(nloc)
    return _NC_CACHE[nloc]


def _make_in_maps(x, Wq, bq, key_prototypes, ncores):
    nloc = x.shape[0] // ncores
    wqt = np.ascontiguousarray(np.asarray(Wq, dtype=np.float32).T)
    kpc = np.ascontiguousarray(np.asarray(key_prototypes, dtype=np.float32))
    bqc = np.ascontiguousarray(
        np.asarray(bq, dtype=np.float32).reshape(INTER, 1)
    )
    xc = np.asarray(x, dtype=np.float32)
    return [
        {
            "x": np.ascontiguousarray(xc[i * nloc : (i + 1) * nloc]),
            "wqt": wqt,
            "kp": kpc,
            "bq": bqc,
        }
        for i in range(ncores)
    ]


def run(inputs, trace: bool = False):
    """Run on hardware; returns (full_output, BassKernelResults)."""
    from concourse.bass_utils import run_bass_kernel_spmd

    x = inputs["x"]
    nloc = x.shape[0] // NCORES
    nc = _get_nc(nloc)
    in_maps = _make_in_maps(
        x, inputs["Wq"], inputs["bq"], inputs["key_prototypes"], NCORES
    )
    res = run_bass_kernel_spmd(nc, in_maps, list(range(NCORES)), trace=trace)
    out = np.concatenate([r["out"] for r in res.results], axis=0)
    return out, res


def kernel(**inputs) -> np.ndarray:
    out, _ = run(inputs, trace=False)
    return out


# revision 4
# speedup vs baseline: 1.0709x; 1.0709x over previous
"""Trainium2 Bass kernel for DifferentiableSparseHypergraph (topk_masking).

Full computation per batch n:
  x_mean = x[n].mean(T)                      (C, V)
  q = Wq @ x_mean + bq                       (O=32, V)   [1x1 conv == matmul]
  q = q / max(||q||_2 over O, eps)
  H_raw = (q^T @ key_prototypes) / sqrt(O)   (V, M=128)
  topk10 -> softmax over the 10 vals -> scatter back; zeros elsewhere.

Kernel strategy (pure data-parallel over batch, 8 cores x 8 batches):
  * t-mean: ONE DVE add level pairs t's (64 -> 32), then the remaining
    32-way t-sum rides the PE: 8 fp32 matmuls (4 free-chunks x 2 c-halves)
    accumulate into a single [32, 512] PSUM tile; slot (tl, v) collects
    t-pairs {tl, 8+tl, 16+tl, 24+tl}. A strided DVE reduce (8 -> 1) and an
    ACT bias/scale step produce q. This moves ~45 us of fp32 tensor_tensor
    work (1x-rate on DVE) onto the otherwise-idle PE.
  * software pipelining: the reduce/bias/score for batch n-1 are EMITTED
    after batch n's adds. DVE executes in order, so putting the reduce
    (which waits ~7 us for the PE matmul group) before the next batch's
    adds would stall the whole add stream behind it.
  * top-10 runs scale-invariantly on the RAW score matmul pb (still in
    PSUM): per-row ordering of H = pb * rn (rn > 0) equals ordering of pb,
    so max/match_replace/max finds the 10th-largest threshold t_k without
    waiting for the norm. exp(H) fuses the rn scale into ACT's Exp; the
    output is exp * (pb >= t_k) / sum -- identical to softmax-over-topk
    scattered back.
  * output DMAs issue from the ACT engine's DGE queue (ot is computed on
    ACT, so the issue is same-engine in-order, no sem) -- an out DMA on
    the sync queue head-of-line-blocks every later x-load issue.
  * batches 0 and 7 stream in t-range chunks so compute starts right as
    the first MiB lands and the tail after the last byte stays short.
"""

import numpy as np

import concourse.bacc as bacc
import concourse.bass as bass
import concourse.mybir as mybir
import concourse.tile as tile

N, C, T, V = 64, 256, 64, 64
INTER = 32          # conv out channels
M = 128             # num hyperedges
TOPK = 10
NCORES = 8
FP = mybir.dt.float32
NEG_BIG = -1.0e30


def build_nc(nloc: int) -> bass.Bass:
    """Build the per-core Bass program processing `nloc` batches."""
    assert nloc % 2 == 0
    # Bacc (not bare Bass): its compile()/finalize() pipeline splits
    # multi-semaphore waits into InstEventSemaphore pairs — walrus allows
    # at most one sync wait per regular instruction.
    nc = bacc.Bacc(target_bir_lowering=False, debug=False)

    x = nc.dram_tensor("x", (nloc, C, T, V), FP, kind="ExternalInput")
    wqt = nc.dram_tensor("wqt", (C, INTER), FP, kind="ExternalInput")
    kp = nc.dram_tensor("kp", (INTER, M), FP, kind="ExternalInput")
    bq = nc.dram_tensor("bq", (INTER, 1), FP, kind="ExternalInput")
    out = nc.dram_tensor("out", (nloc, V, M), FP, kind="ExternalOutput")

    A = mybir.AluOpType
    AF = mybir.ActivationFunctionType
    from concourse.tile import add_dep_helper

    last = nloc - 1

    with tile.TileContext(nc) as tc:
        with (
            tc.tile_pool(name="consts", bufs=1) as consts,
            tc.tile_pool(name="xph", bufs=3) as xph,
            tc.tile_pool(name="xp", bufs=2) as xp,
            tc.tile_pool(name="small", bufs=2) as small,
            tc.tile_pool(name="psA", bufs=2, space="PSUM") as psA,
            tc.tile_pool(name="psB", bufs=2, space="PSUM") as psB,
            tc.tile_pool(name="psS", bufs=1, space="PSUM") as psS,
        ):
            # --- batch 0's x loads go FIRST (before the const DMAs) and in
            # 1 MiB t-range chunks, so the DVE adds start as early as
            # possible.  Chunked batches pair t with t+1 (each chunk is
            # self-contained); whole-half batches pair t with t+32.
            xh0 = [
                xph.tile([128, T * V], FP, tag=f"xh{h}", name=f"xh0_{h}")
                for h in range(2)
            ]
            for h in range(2):
                for c in range(2):
                    nc.sync.dma_start(
                        out=xh0[h][:, c * 2048 : (c + 1) * 2048],
                        in_=x[0, h * 128 : (h + 1) * 128,
                              c * (T // 2) : (c + 1) * (T // 2)],
                    )

            # --- replicated constants ---
            wq_sb = consts.tile([128, 2, INTER], FP)    # [c, c_half, o]
            nc.sync.dma_start(
                out=wq_sb[:], in_=wqt.rearrange("(h c) o -> c h o", h=2)
            )
            kp_sb = consts.tile([INTER, M], FP)
            nc.sync.dma_start(out=kp_sb[:], in_=kp[:])
            bq_sb = consts.tile([INTER, 1], FP)
            nc.sync.dma_start(out=bq_sb[:], in_=bq[:])
            ones_sb = consts.tile([INTER, 1], FP)
            nc.vector.memset(ones_sb[:], 1.0)

            # The fp32 self-loading matmul can carry at most ONE semaphore
            # wait (walrus S3_LW_STRUCT limit). Absorb the wq/kp DMA waits
            # with dummy 1x1 matmuls so the first real matmuls only wait on
            # their a1-tile DVE sem.
            scr = psS.tile([1, 1], FP)
            d1 = nc.tensor.matmul(
                scr[:], wq_sb[:, 0, 0:1], wq_sb[:, 0, 0:1], start=True, stop=True
            )
            d2 = nc.tensor.matmul(
                scr[:], kp_sb[:, 0:1], kp_sb[:, 0:1], start=True, stop=True
            )
            add_dep_helper(d2.ins, d1.ins, sync=False, reason="pe-wait-absorb order")

            q2 = {}          # pair -> q2 tile
            pending = None   # (n, pa) awaiting reduce/bias/score
            first_mm = None

            def finish(n, pa):
                """Emit reduce + bias for batch n; score + out for odd n."""
                l = n % 2
                p = n // 2
                qtmp = small.tile([INTER, V], FP, tag="qtmp")
                nc.vector.reduce_sum(
                    out=qtmp[:],
                    in_=pa[:].rearrange("o (t v) -> o v t", t=8),
                    axis=mybir.AxisListType.X,
                )
                nc.scalar.activation(
                    q2[p][:, l * V : (l + 1) * V],
                    qtmp[:],
                    AF.Identity,
                    bias=bq_sb[:],
                    scale=1.0 / T,
                )
                if l == 0:
                    return

                # raw scores pb[vv, m] = q2^T . kp stay in PSUM; the top-10
                # threshold is found on pb directly (ordering-invariant to
                # the positive per-row rescale rn).
                qsq = small.tile([INTER, 2 * V], FP, tag="qsq")
                nc.scalar.activation(qsq[:], q2[p][:], AF.Square)
                pb = psB.tile([2 * V, M], FP, tag="pb")
                nc.tensor.matmul(pb[:], q2[p][:], kp_sb[:], start=True, stop=True)
                pc = psB.tile([2 * V, 1], FP, tag="pc")
                nc.tensor.matmul(pc[:], qsq[:], ones_sb[:], start=True, stop=True)
                # rn = 1/sqrt(INTER * nsq) = INTER^-0.5 / ||q||
                nrm = small.tile([2 * V, 1], FP, tag="nrm")
                nc.scalar.activation(nrm[:], pc[:], AF.Sqrt, scale=float(INTER))
                rn = small.tile([2 * V, 1], FP, tag="rn")
                nc.vector.reciprocal(rn[:], nrm[:])

                # t_k = 10th largest per row: top8, knock out, top8 again
                top8a = small.tile([2 * V, 8], FP, tag="t8a")
                nc.vector.max(top8a[:], pb[:])
                work = small.tile([2 * V, M], FP, tag="work")
                nc.vector.match_replace(work[:], top8a[:], pb[:], NEG_BIG)
                top8b = small.tile([2 * V, 8], FP, tag="t8b")
                nc.vector.max(top8b[:], work[:])

                # e = exp(H) = exp(pb * rn)  (rn fused into ACT's scale);
                # masked softmax without scatter:
                # me = (pb >= t_k) * e; out = me / sum(me)
                e = small.tile([2 * V, M], FP, tag="e")
                nc.scalar.activation(e[:], pb[:], AF.Exp, scale=rn[:])
                me = small.tile([2 * V, M], FP, tag="me")
                s = small.tile([2 * V, 1], FP, tag="s")
                nc.vector.scalar_tensor_tensor(
                    out=me[:],
                    in0=pb[:],
                    scalar=top8b[:, 1:2],
                    in1=e[:],
                    op0=A.is_ge,
                    op1=A.mult,
                    accum_out=s[:],
                )
                r = small.tile([2 * V, 1], FP, tag="r")
                nc.vector.reciprocal(r[:], s[:])
                ot = small.tile([2 * V, M], FP, tag="ot")
                nc.scalar.activation(ot[:], me[:], AF.Copy, scale=r[:])

                # ACT-queue DMA: same-engine in-order after ot, and keeps
                # the sync queue free for x-load issues.
                nc.scalar.dma_start(
                    out=out[2 * p : 2 * p + 2].rearrange("b v m -> (b v) m"),
                    in_=ot[:],
                )

            for n in range(nloc):
                chunked = n == 0 or n == last
                if n == 0:
                    xh = xh0
                else:
                    xh = []
                    for h in range(2):
                        t = xph.tile([128, T * V], FP, tag=f"xh{h}")
                        if chunked:
                            # last batch: 4 t-range chunks per half so the
                            # post-last-byte tail is one chunk, not a half
                            for c in range(4):
                                nc.sync.dma_start(
                                    out=t[:, c * 1024 : (c + 1) * 1024],
                                    in_=x[n, h * 128 : (h + 1) * 128,
                                          c * (T // 4) : (c + 1) * (T // 4)],
                                )
                        else:
                            nc.sync.dma_start(
                                out=t[:], in_=x[n, h * 128 : (h + 1) * 128]
                            )
                        xh.append(t)

                if n % 2 == 0:
                    q2[n // 2] = small.tile(
                        [INTER, 2 * V], FP, tag="q2", name=f"q2_{n // 2}"
                    )

                # one DVE add level: t 64 -> 32 (a1 free = 32 t-slots x V),
                # then 8 accumulating matmuls fold the 32 slots into 8 PSUM
                # slots (tl, v) while contracting c.
                pa = psA.tile([INTER, 512], FP, tag="pa")
                mm_idx = 0
                for h in range(2):
                    a1 = xp.tile([128, T * V // 2], FP, tag=f"a1{h}")
                    if chunked:
                        nch = 2 if n == 0 else 4
                        csz = 4096 // nch
                        for c in range(nch):
                            src = xh[h][
                                :, c * csz : (c + 1) * csz
                            ].rearrange("p (t two v) -> p t two v", two=2, v=V)
                            dst = a1[
                                :, c * (csz // 2) : (c + 1) * (csz // 2)
                            ].rearrange("p (t v) -> p t v", v=V)
                            nc.vector.tensor_add(
                                dst, src[:, :, 0, :], src[:, :, 1, :]
                            )
                            # matmul each completed 512-span of a1 right away
                            done = (c + 1) * (csz // 2)
                            while mm_idx * 512 + 512 <= h * 2048 + done:
                                j = mm_idx - h * 4
                                mm = nc.tensor.matmul(
                                    pa[:],
                                    wq_sb[:, h, :],
                                    a1[:, j * 512 : (j + 1) * 512],
                                    start=(mm_idx == 0),
                                    stop=(mm_idx == 7),
                                )
                                if first_mm is None:
                                    first_mm = mm
                                    add_dep_helper(
                                        mm.ins, d2.ins, sync=False,
                                        reason="pe-wait-absorb order",
                                    )
                                mm_idx += 1
                    else:
                        nc.vector.tensor_add(
                            a1[:],
                            xh[h][:, : T * V // 2],
                            xh[h][:, T * V // 2 :],
                        )
                        for j in range(4):
                            nc.tensor.matmul(
                                pa[:],
                                wq_sb[:, h, :],
                                a1[:, j * 512 : (j + 1) * 512],
                                start=(mm_idx == 0),
                                stop=(mm_idx == 7),
                            )
                            mm_idx += 1

                # delayed by one batch: reduce/bias/score of batch n-1
                if pending is not None:
                    finish(*pending)
                pending = (n, pa)

            finish(*pending)
    nc.finalize()
    return nc


_NC_CACHE: dict[int, bass.Bass] = {}


def _get_nc(nloc: int) -> bass.Bass:
    if nloc not in _NC_CACHE:
        _NC_CACHE[nloc] = build_nc(nloc)
    return _NC_CACHE[nloc]


def _make_in_maps(x, Wq, bq, key_prototypes, ncores):
    nloc = x.shape[0] // ncores
    wqt = np.ascontiguousarray(np.asarray(Wq, dtype=np.float32).T)
    kpc = np.ascontiguousarray(np.asarray(key_prototypes, dtype=np.float32))
    bqc = np.ascontiguousarray(
        np.asarray(bq, dtype=np.float32).reshape(INTER, 1)
    )
    xc = np.asarray(x, dtype=np.float32)
    return [
        {
            "x": np.ascontiguousarray(xc[i * nloc : (i + 1) * nloc]),
            "wqt": wqt,
            "kp": kpc,
            "bq": bqc,
        }
        for i in range(ncores)
    ]


def run(inputs, trace: bool = False):
    """Run on hardware; returns (full_output, BassKernelResults)."""
    from concourse.bass_utils import run_bass_kernel_spmd

    x = inputs["x"]
    nloc = x.shape[0] // NCORES
    nc = _get_nc(nloc)
    in_maps = _make_in_maps(
        x, inputs["Wq"], inputs["bq"], inputs["key_prototypes"], NCORES
    )
    res = run_bass_kernel_spmd(nc, in_maps, list(range(NCORES)), trace=trace)
    out = np.concatenate([r["out"] for r in res.results], axis=0)
    return out, res


def kernel(**inputs) -> np.ndarray:
    out, _ = run(inputs, trace=False)
    return out
